# revision 4
# baseline (speedup 1.0000x reference)
"""Sparse graph-attention kernel for 8 TRN2 NeuronCores (Bass/Tile).

Problem (hardcoded): N=20000 nodes, E=640000 edges (src-sorted), Fin=256,
Fqk=256.  out[e] = exp(aw[e]) / segsum_src(exp(aw)),
aw[e] = (x[src[e]] @ Wq.T * Fqk**-0.5) . (x[dest[e]] @ Wk.T).

Sharding: src-node ranges (2500 nodes/core); per-core inputs are rotated so
all cores run one identical static graph.  Per-node edge lists are padded to
multiples of 8 slots ("subrows") and packed into 128 partitions x BLK
subrow-blocks, nodes never straddling a partition.  k/q row tables are
computed on device (bf16), per-edge rows fetched with dma_gather, dots done
with DVE scalar_tensor_tensor accumulate, segment softmax with two
tensor_tensor_scans (fwd segmented sum + reverse max-propagate).
"""

import numpy as np
import ml_dtypes

N = 20000
E = 640000
FIN = 256
FQK = 256
NCORES = 8
NLOC = N // NCORES          # 2500 nodes per core
SLOTS = 8                   # slots per subrow
BLK = 92                    # subrow blocks per partition
P = 128
RCAP = P * BLK              # 11776 subrows per core (capacity)
NSLOT = RCAP * SLOTS        # 94208 gather slots per core
NPAD = 20096                # 157*128, padded node count for matmul
QROWS = 2560                # 20*128, q table rows
NT = NPAD // P              # 157 node tiles
QT = QROWS // P             # 20 q-covering node tiles
GCH = 2944                  # gather chunk (idxs per dma_gather) = 23*128
GBLK = GCH // P             # 23 subrow blocks per gather chunk
NKCH = NSLOT // GCH         # 32 k-gather chunks (4 per slot region)
NQCH = RCAP // GCH          # 4 q-gather chunks
SEG = BLK * SLOTS           # 736 free positions per partition

bf16 = ml_dtypes.bfloat16
_compiled = None            # (nc, names) cache


def _wrap_idx(vals):
    """int16 vals [n] (n % GCH == 0) -> wrapped dma_gather layout [128, n/16]:
    per 2944-chunk, idx j -> partition j%16 (replicated 8x), col j//16."""
    n = vals.shape[0]
    nch = n // GCH
    a = vals.reshape(nch, GCH // 16, 16).transpose(0, 2, 1)      # [nch,16,184]
    a = np.broadcast_to(a[:, None, :, :], (nch, 8, 16, GCH // 16))
    return np.ascontiguousarray(
        a.reshape(nch, 128, GCH // 16).transpose(1, 0, 2).reshape(128, n // 16)
    )


def _host_prep(x, ei, W):
    src = np.asarray(ei[0], np.int64)
    dest = np.asarray(ei[1], np.int64)
    x = np.asarray(x, np.float32)
    W = np.asarray(W, np.float32)

    # W^T with softmax scaling folded into the Wq half.
    Ws = W.copy()
    Ws[:FQK] *= FQK ** -0.5
    wT = np.ascontiguousarray(Ws.T.astype(bf16))                 # [256, 512]

    xb = x.astype(bf16)                                          # [N, 256]

    counts = np.bincount(src, minlength=N)
    starts = np.concatenate([[0], np.cumsum(counts)])            # [N+1]

    in_maps = []
    unshard = []
    for c in range(NCORES):
        n0 = c * NLOC
        # per-node subrow packing into partitions (none straddles a row)
        eg = np.full((P, BLK, SLOTS), -1, np.int64)              # edge ids
        seg_start = np.zeros((P, SEG), bool)
        seg_end = np.zeros((P, SEG), bool)
        p, b = 0, 0
        for n in range(n0, n0 + NLOC):
            d = int(counts[n])
            if d == 0:
                continue
            nsub = (d + SLOTS - 1) // SLOTS
            if b + nsub > BLK:
                if b < BLK:          # pad tail of this partition = 1 segment
                    seg_start[p, b * SLOTS] = True
                    seg_end[p, SEG - 1] = True
                p += 1
                b = 0
                assert p < P, "subrow capacity exceeded"
            e0 = starts[n]
            flat = eg[p, b:b + nsub].reshape(-1)
            flat[:d] = np.arange(e0, e0 + d)
            eg[p, b:b + nsub] = flat.reshape(nsub, SLOTS)
            seg_start[p, b * SLOTS] = True
            seg_end[p, (b + nsub) * SLOTS - 1] = True
            b += nsub
        if b < BLK:
            seg_start[p, b * SLOTS] = True
            seg_end[p, SEG - 1] = True
        for pp in range(p + 1, P):   # fully-pad partitions
            seg_start[pp, 0] = True
            seg_end[pp, SEG - 1] = True

        # gather index values (rotated node ids)
        egT = eg.transpose(2, 1, 0)                              # [8, 92, 128]
        valid = egT >= 0
        e_ids = np.where(valid, egT, 0)
        kval = np.where(valid, (dest[e_ids] - n0) % N, 0).astype(np.int16)
        kg_idx = _wrap_idx(kval.reshape(-1))                     # [128, 5888]

        eg0 = eg[:, :, 0].transpose(1, 0)                        # [92, 128]
        v0 = eg0 >= 0
        qval = np.where(v0, src[np.where(v0, eg0, 0)] - n0, 0).astype(np.int16)
        qg_idx = _wrap_idx(qval.reshape(-1))                     # [128, 736]

        maskB = np.where(eg.reshape(P, SEG) >= 0, 0.0, -30.0).astype(np.float32)
        cmask = np.where(seg_start, 0.0, 1.0).astype(np.float32)
        emask_rev = np.ascontiguousarray(
            np.where(seg_end, 0.0, 1.0).astype(np.float32)[:, ::-1])

        xT = np.zeros((FIN, NPAD), bf16)
        xr = np.concatenate([xb[n0:], xb[:n0]], axis=0)          # rotated
        xT[:, :N] = xr.T
        xT = np.ascontiguousarray(xT)

        in_maps.append(dict(xT=xT, wT=wT, kg_idx=kg_idx, qg_idx=qg_idx,
                            maskB=maskB, cmask=cmask, emask_rev=emask_rev))

        pm, bm, sm = np.where(eg >= 0)
        unshard.append((eg[pm, bm, sm], pm, bm * SLOTS + sm))
    return in_maps, unshard


def _build():
    import concourse.bacc as bacc
    import concourse.mybir as mybir
    import concourse.tile as tile
    from concourse import library_config
    from concourse.tile_rust import add_dep_helper

    fp32 = mybir.dt.float32
    b16 = mybir.dt.bfloat16
    Alu = mybir.AluOpType

    nc = bacc.Bacc("TRN2", target_bir_lowering=False, debug=False)
    xT_d = nc.dram_tensor("xT", [FIN, NPAD], b16, kind="ExternalInput")
    wT_d = nc.dram_tensor("wT", [FIN, 2 * FQK], b16, kind="ExternalInput")
    kgi_d = nc.dram_tensor("kg_idx", [P, NSLOT // 16], mybir.dt.int16,
                           kind="ExternalInput")
    qgi_d = nc.dram_tensor("qg_idx", [P, RCAP // 16], mybir.dt.int16,
                           kind="ExternalInput")
    mB_d = nc.dram_tensor("maskB", [P, SEG], fp32, kind="ExternalInput")
    cm_d = nc.dram_tensor("cmask", [P, SEG], fp32, kind="ExternalInput")
    em_d = nc.dram_tensor("emask_rev", [P, SEG], fp32, kind="ExternalInput")
    out_d = nc.dram_tensor("out", [P, SEG], fp32, kind="ExternalOutput")

    with tile.TileContext(nc) as tc:
        with tc.tile_pool(name="dram", bufs=1, space="DRAM") as dram, \
             tc.tile_pool(name="persist", bufs=1) as sb, \
             tc.tile_pool(name="psum", bufs=4, space="PSUM") as ps:
            k_tab = dram.tile([NPAD, FQK], b16)          # k row table
            q_tab = dram.tile([QROWS, FQK], b16)         # q row table

            lib = nc.gpsimd.load_library(library_config.mlp)

            # --- input loads ---
            wt = sb.tile([P, 2, 2 * FQK], b16)
            nc.sync.dma_start(wt[:], wT_d[:, :].rearrange("(c p) f -> p c f", p=P))
            kgi = sb.tile([P, NSLOT // 16], mybir.dt.int16)
            nc.sync.dma_start(kgi[:], kgi_d[:])
            qgi = sb.tile([P, RCAP // 16], mybir.dt.int16)
            nc.sync.dma_start(qgi[:], qgi_d[:])
            mB = sb.tile([P, SEG], fp32)
            nc.sync.dma_start(mB[:], mB_d[:])
            cm = sb.tile([P, SEG], fp32)
            nc.sync.dma_start(cm[:], cm_d[:])
            em = sb.tile([P, SEG], fp32)
            nc.sync.dma_start(em[:], em_d[:])

            with tc.tile_pool(name="xt", bufs=1) as xtp, \
                 tc.tile_pool(name="qk", bufs=4) as qksb:
                xTs = xtp.tile([P, 2, NPAD], b16)
                XCH = 2512
                for j in range(NPAD // XCH):
                    sl = slice(j * XCH, (j + 1) * XCH)
                    nc.sync.dma_start(
                        xTs[:, :, sl],
                        xT_d[:, sl].rearrange("(c p) f -> p c f", p=P))

                # --- q/k projection: for each 128-node tile ---
                for nt in range(NT):
                    sl = slice(nt * P, (nt + 1) * P)
                    acc = ps.tile([P, 2 * FQK], fp32)
                    nc.tensor.matmul(acc[:], lhsT=xTs[:, 0, sl], rhs=wt[:, 0, :],
                                     start=True, stop=False)
                    nc.tensor.matmul(acc[:], lhsT=xTs[:, 1, sl], rhs=wt[:, 1, :],
                                     start=False, stop=True)
                    qk = qksb.tile([P, 2 * FQK], b16, tag="qk")
                    if nt % 2 == 0:
                        nc.scalar.copy(qk[:], acc[:])
                    else:
                        nc.vector.tensor_copy(qk[:], acc[:])
                    nc.sync.dma_start(k_tab[sl, :], qk[:, FQK:])
                    if nt < QT:
                        nc.sync.dma_start(q_tab[sl, :], qk[:, :FQK])

            # --- q gather (per subrow) ---
            qg = sb.tile([P, BLK, FQK], b16)
            for j in range(NQCH):
                g = nc.gpsimd.dma_gather(
                    qg[:, j * GBLK:(j + 1) * GBLK, :], q_tab[:],
                    qgi[:, j * (GCH // 16):(j + 1) * (GCH // 16)],
                    GCH, GCH, FQK, single_packet=False)
                add_dep_helper(lib.ins, g.ins, sync=True, reason="lib first")

            # --- k gather + dots ---
            aw = sb.tile([P, SEG], fp32)
            with tc.tile_pool(name="kg", bufs=4) as kgsb, \
                 tc.tile_pool(name="scr", bufs=4) as scrsb:
                for s in range(SLOTS):
                    for j in range(NQCH):
                        ci = s * NQCH + j
                        kg = kgsb.tile([P, GBLK, FQK], b16, tag="kg")
                        g = nc.gpsimd.dma_gather(
                            kg[:], k_tab[:],
                            kgi[:, ci * (GCH // 16):(ci + 1) * (GCH // 16)],
                            GCH, GCH, FQK, single_packet=False)
                        add_dep_helper(lib.ins, g.ins, sync=True, reason="lib first")
                        for b in range(GBLK):
                            blk = j * GBLK + b
                            scr = scrsb.tile([P, FQK], b16, tag="scr")
                            nc.vector.scalar_tensor_tensor(
                                out=scr[:], in0=kg[:, b, :], scalar=1.0,
                                in1=qg[:, blk, :],
                                op0=Alu.mult, op1=Alu.mult,
                                accum_out=aw[:, blk * SLOTS + s:blk * SLOTS + s + 1])

            # --- segment softmax ---
            with tc.tile_pool(name="smx", bufs=1) as smx:
                awm = smx.tile([P, SEG], fp32)
                nc.vector.tensor_tensor(out=awm[:], in0=aw[:], in1=mB[:], op=Alu.add)
                ex = smx.tile([P, SEG], fp32)
                nc.scalar.activation(ex[:], awm[:], mybir.ActivationFunctionType.Exp)
                pfx = smx.tile([P, SEG], fp32)
                nc.vector.tensor_tensor_scan(
                    out=pfx[:], data0=cm[:], data1=ex[:], initial=0.0,
                    op0=Alu.mult, op1=Alu.add)
                tot = smx.tile([P, SEG], fp32)
                nc.vector.tensor_tensor_scan(
                    out=tot[:, ::-1], data0=em[:], data1=pfx[:, ::-1], initial=0.0,
                    op0=Alu.mult, op1=Alu.max)
                rec = smx.tile([P, SEG], fp32)
                nc.vector.reciprocal(rec[:], tot[:])
                o = smx.tile([P, SEG], fp32)
                nc.vector.tensor_tensor(out=o[:], in0=ex[:], in1=rec[:], op=Alu.mult)
                nc.sync.dma_start(out_d[:], o[:])
    nc.compile()
    return nc


def kernel(x, ei, W):
    global _compiled
    in_maps, unshard = _host_prep(x, ei, W)
    if _compiled is None:
        _compiled = _build()
    nc = _compiled
    from concourse.bass_utils import run_bass_kernel_spmd
    res = run_bass_kernel_spmd(nc, in_maps, core_ids=list(range(NCORES)))
    out = np.empty(E, np.float32)
    for c in range(NCORES):
        eids, pm, tm = unshard[c]
        out[eids] = res.results[c]["out"][pm, tm]
    return out
